# revision 2
# baseline (speedup 1.0000x reference)
"""Chamfer distance (mean of sqrt of min squared distances, both directions) on 8
Trainium2 NeuronCores.

Strategy
--------
Data-parallel over the batch dim: core b handles batch b (B=8, one batch per core).

Per batch, both clouds are sorted (host) along coordinate 0. For each query point a
certified NN search window into the sorted target cloud is computed on host via a
witness bound: r_i = distance to the best of 64 z-nearest candidates; any target
with |z_t - z_q| > r_i is provably farther than the witness, so the true NN lies
inside the window. Points with extreme windows (> threshold T, chosen by exact
cost search on the data) are gathered into separate "overflow" blocks. Window
extents are unioned per block and across batches (one compiled program serves all
cores). Each block's window is covered by 512-wide chunks, grouped <=4 chunks per
PSUM tile; multi-group blocks get temp result columns reduced at the end.

On device, squared distances are computed on the TensorEngine as K=16 matmuls in
bf16 hi/lo split form (abs error ~1e-6):
    g[m,n] = -2*q_m . t_n + |t_n|^2   (4-way bf16 split of |t|^2; all norms from
                                       the bf16-reconstructed points, so g + |q|^2
                                       is exactly |q_hat - t_hat|^2)
    d[m,n] = g[m,n] + |q_m|^2         (query norm added post-min in fp32)
Row minima are reduced on the VectorEngine straight out of PSUM; sqrt runs on the
ScalarEngine with fused row-sum accumulation; a ones-vector matmul does the final
partition sum. Host averages the 16 per-core sums.
"""

import numpy as np
import ml_dtypes

bf = ml_dtypes.bfloat16

B, N, D = 8, 8192, 3
BLK = 128             # query block (matmul M)
CH = 512              # window chunk (matmul N)
NBLK = N // BLK       # 64 regular blocks
GRP = 4               # max chunks per PSUM tile / virtual group
C_WIT = 64            # witness candidates
EPS = 1e-12
T_GRID = (768, 1024, 1536, 2048, 3072, 4096, 6144, 9000)


# ---------------------------------------------------------------- host-side prep
def _splitk(a, k):
    out = []
    r = np.asarray(a, np.float64)
    for _ in range(k):
        h = r.astype(bf)
        out.append(h)
        r = r - h.astype(np.float64)
    return out


def _build_LR(p):
    """p [n,3] f32 sorted -> L [16,n] bf16, R [16,n] bf16, q2 [n] f32."""
    p64 = p.astype(np.float64)
    ph, pl = _splitk(p64, 2)
    phat = ph.astype(np.float64) + pl.astype(np.float64)
    m2h = [(-2.0 * ph[:, c].astype(np.float64)).astype(bf) for c in range(3)]
    m2l = [(-2.0 * pl[:, c].astype(np.float64)).astype(bf) for c in range(3)]
    t2 = (phat ** 2).sum(1)
    t2s = _splitk(t2, 4)
    one = np.ones(p.shape[0], bf)
    L = np.stack(m2h + m2h + m2l + m2l + [one, one, one, one], 0)
    R = np.stack([ph[:, 0], ph[:, 1], ph[:, 2], pl[:, 0], pl[:, 1], pl[:, 2],
                  ph[:, 0], ph[:, 1], ph[:, 2], pl[:, 0], pl[:, 1], pl[:, 2],
                  t2s[0], t2s[1], t2s[2], t2s[3]], 0)
    return L, R, t2.astype(np.float32)


def _pp_windows(q, t):
    """Certified per-point NN windows of q into sorted t -> (lo, hi) int64."""
    zq = q[:, 0].astype(np.float64)
    pos = np.searchsorted(t[:, 0], q[:, 0])
    lo_c = np.clip(pos - C_WIT // 2, 0, N - C_WIT)
    idx = lo_c[:, None] + np.arange(C_WIT)[None, :]
    d = ((q[:, None, :].astype(np.float64) - t[idx].astype(np.float64)) ** 2).sum(-1)
    r = np.sqrt(d.min(1)) * (1 + 1e-6) + 1e-7
    lo = np.searchsorted(t[:, 0], zq - r, side="left")
    hi = np.searchsorted(t[:, 0], zq + r, side="right")
    return lo, hi


def _chunks_for(lo, hi):
    """512-aligned chunk starts covering [lo, hi), clamped into [0, N-CH]."""
    lo = int(min(max(lo, 0), N - 1))
    hi = int(max(hi, lo + 1))
    nch = max(1, -(-(hi - lo) // CH))
    return [min(lo + CH * c, N - CH) for c in range(nch)]


def _schedule_dir(LOd, HId, T):
    """LOd/HId: [B, N] per-point windows for one direction.
    Returns (blocks, nob, cols) where blocks is a list over block_id of
    dicts {groups: [[starts...]...], final_col, temp_cols(range)} and the
    gathered overflow index arrays per batch (ovf_idx [B, nob*BLK])."""
    WID = HId - LOd
    ovf = WID > T                              # [B, N]
    nob = int(np.ceil(max(1, ovf.sum(1).max()) / BLK)) if ovf.any() else 0

    # gathered overflow membership, padded with last real (or 0)
    ovf_idx = np.zeros((B, max(nob, 0) * BLK), np.int64)
    ovf_cnt = np.zeros(B, np.int64)
    for b in range(B):
        ix = np.where(ovf[b])[0]
        ovf_cnt[b] = len(ix)
        if nob:
            padv = ix[-1] if len(ix) else 0
            pad = np.full(nob * BLK, padv, np.int64)
            pad[: len(ix)] = ix
            ovf_idx[b] = pad

    nblk_tot = NBLK + nob
    # per-block unions across batches
    uni = []
    for k in range(NBLK):
        sl = slice(k * BLK, (k + 1) * BLK)
        lo_m = np.where(ovf[:, sl], N, LOd[:, sl]).min()
        hi_m = np.where(ovf[:, sl], 0, HId[:, sl]).max()
        uni.append((lo_m, hi_m))
    for o in range(nob):
        sl = slice(o * BLK, (o + 1) * BLK)
        lo_m, hi_m = N, 0
        for b in range(B):
            ix = ovf_idx[b, sl]
            lo_m = min(lo_m, LOd[b, ix].min())
            hi_m = max(hi_m, HId[b, ix].max())
        uni.append((lo_m, hi_m))

    blocks = []
    ntemp = 0
    for bid in range(nblk_tot):
        starts = _chunks_for(*uni[bid])
        groups = [starts[i: i + GRP] for i in range(0, len(starts), GRP)]
        blocks.append({"groups": groups})
    # column layout: finals [0, nblk_tot), temps afterwards
    tc = nblk_tot
    for blk in blocks:
        g = len(blk["groups"])
        if g == 1:
            blk["temp"] = None
        else:
            blk["temp"] = (tc, tc + g)
            tc += g
    ncol = tc
    return blocks, nob, ncol, ovf_idx, ovf_cnt, ovf


def _dir_cost(blocks):
    cols = 0
    ops = 0
    for blk in blocks:
        for g in blk["groups"]:
            cols += len(g) * CH
            ops += 1
        if blk["temp"] is not None:
            ops += 1
    return cols + 170 * ops


def _prepare(xyz1, xyz2):
    xs_l, ys_l = [], []
    for b in range(B):
        x = np.asarray(xyz1[b], np.float32)
        y = np.asarray(xyz2[b], np.float32)
        xs_l.append(x[np.argsort(x[:, 0], kind="stable")])
        ys_l.append(y[np.argsort(y[:, 0], kind="stable")])

    LO = np.zeros((B, 2, N), np.int64)
    HI = np.zeros((B, 2, N), np.int64)
    for b in range(B):
        for di, (q, t) in enumerate([(xs_l[b], ys_l[b]), (ys_l[b], xs_l[b])]):
            lo, hi = _pp_windows(q, t)
            LO[b, di], HI[b, di] = lo, hi

    dirs = []
    for di in range(2):
        best = None
        for T in T_GRID:
            cand = _schedule_dir(LO[:, di], HI[:, di], T)
            c = _dir_cost(cand[0])
            if best is None or c < best[0]:
                best = (c, T, cand)
        _, T, (blocks, nob, ncol, ovf_idx, ovf_cnt, ovf) = best
        dirs.append({"T": T, "blocks": blocks, "nob": nob, "ncol": ncol,
                     "ovf_idx": ovf_idx, "ovf_cnt": ovf_cnt, "ovf": ovf})

    # ---- per-core tensors
    in_maps = []
    for b in range(B):
        Lx, Rx, q2x = _build_LR(xs_l[b])
        Ly, Ry, q2y = _build_LR(ys_l[b])
        aug_parts = [Lx, Ly, Ry, Rx]
        q2_parts, mask_parts = [], []
        for di in range(2):
            dd = dirs[di]
            q2q = (q2x, q2y)[di]
            Lq = (Lx, Ly)[di]
            nob, ncol = dd["nob"], dd["ncol"]
            if nob:
                aug_parts.append(Lq[:, dd["ovf_idx"][b]])       # [16, nob*BLK]
            q2c = np.zeros((BLK, ncol), np.float32)
            mkc = np.zeros((BLK, ncol), np.float32)
            q2c[:, :NBLK] = q2q.reshape(NBLK, BLK).T
            mkc[:, :NBLK] = (~dd["ovf"][b]).astype(np.float32).reshape(NBLK, BLK).T
            for o in range(nob):
                ix = dd["ovf_idx"][b, o * BLK: (o + 1) * BLK]
                q2c[:, NBLK + o] = q2q[ix]
                slot = o * BLK + np.arange(BLK)
                mkc[:, NBLK + o] = (slot < dd["ovf_cnt"][b]).astype(np.float32)
            q2_parts.append(q2c)
            mask_parts.append(mkc)
        aug = np.concatenate(aug_parts, 1).astype(bf)
        in_maps.append({"aug": np.ascontiguousarray(aug),
                        "q2": np.ascontiguousarray(
                            np.concatenate(q2_parts, 1).astype(np.float32)),
                        "mask": np.ascontiguousarray(
                            np.concatenate(mask_parts, 1).astype(np.float32))})
    return in_maps, dirs


# ---------------------------------------------------------------- device kernel
def _schedule_key(dirs):
    key = []
    for dd in dirs:
        key.append((dd["nob"], dd["ncol"],
                    tuple(tuple(tuple(g) for g in blk["groups"]) +
                          (blk["temp"],) for blk in dd["blocks"])))
    return tuple(key)


def _build_nc(dirs, repeat=1):
    import concourse.bacc as bacc
    import concourse.tile as tile
    import concourse.mybir as mybir

    F32 = mybir.dt.float32
    BF16 = mybir.dt.bfloat16
    AX = mybir.AxisListType.X
    MIN = mybir.AluOpType.min
    ADD = mybir.AluOpType.add
    MUL = mybir.AluOpType.mult

    K = 16
    LBASE = [0, N]
    RBASE = [2 * N, 3 * N]
    OBASE = [4 * N, 4 * N + BLK * dirs[0]["nob"]]
    AUGW = 4 * N + BLK * (dirs[0]["nob"] + dirs[1]["nob"])
    NCOL = dirs[0]["ncol"] + dirs[1]["ncol"]
    CBASE = [0, dirs[0]["ncol"]]

    nc = bacc.Bacc("TRN2", target_bir_lowering=False, debug=False)
    aug_d = nc.dram_tensor("aug", [K, AUGW], BF16, kind="ExternalInput").ap()
    q2_d = nc.dram_tensor("q2", [BLK, NCOL], F32, kind="ExternalInput").ap()
    mask_d = nc.dram_tensor("mask", [BLK, NCOL], F32, kind="ExternalInput").ap()
    out_d = nc.dram_tensor("out", [1, 2], F32, kind="ExternalOutput").ap()

    with tile.TileContext(nc) as tc:
        with (
            tc.tile_pool(name="cst", bufs=1) as cst,
            tc.tile_pool(name="work", bufs=2) as work,
            tc.tile_pool(name="ps", bufs=2, space="PSUM") as ps,
        ):
            aug_t = cst.tile([128, AUGW], BF16)
            NDMA = 8
            step = -(-AUGW // NDMA)
            for i in range(NDMA):
                s = i * step
                e = min(AUGW, s + step)
                if s < e:
                    nc.sync.dma_start(aug_t[0:K, s:e], aug_d[:, s:e])
            q2_t = cst.tile([128, NCOL], F32)
            nc.sync.dma_start(q2_t[:, :], q2_d[:, :])
            mask_t = cst.tile([128, NCOL], F32)
            nc.sync.dma_start(mask_t[:, :], mask_d[:, :])
            ones_t = cst.tile([128, 1], F32)
            nc.vector.memset(ones_t, 1.0)

            for _rep in range(repeat):
                rowmin = work.tile([128, NCOL], F32, tag="rowmin")
                sums = work.tile([128, 2], F32, tag="sums")
                for di in range(2):
                    dd = dirs[di]
                    cb = CBASE[di]
                    for bid, blk in enumerate(dd["blocks"]):
                        if bid < NBLK:
                            lhs = aug_t[0:K, LBASE[di] + BLK * bid:
                                        LBASE[di] + BLK * (bid + 1)]
                        else:
                            o = bid - NBLK
                            lhs = aug_t[0:K, OBASE[di] + BLK * o:
                                        OBASE[di] + BLK * (o + 1)]
                        groups = blk["groups"]
                        for gi, starts in enumerate(groups):
                            nch = len(starts)
                            dps = ps.tile([128, GRP * CH], F32, tag="d")
                            for c, s in enumerate(starts):
                                nc.tensor.matmul(
                                    dps[:, c * CH:(c + 1) * CH],
                                    lhs,
                                    aug_t[0:K, RBASE[di] + s: RBASE[di] + s + CH],
                                    start=True, stop=True,
                                )
                            if blk["temp"] is None:
                                ocol = cb + bid
                            else:
                                ocol = cb + blk["temp"][0] + gi
                            nc.vector.tensor_reduce(
                                out=rowmin[:, ocol: ocol + 1],
                                in_=dps[:, 0:nch * CH], axis=AX, op=MIN,
                            )
                        if blk["temp"] is not None:
                            t0, t1 = blk["temp"]
                            nc.vector.tensor_reduce(
                                out=rowmin[:, cb + bid: cb + bid + 1],
                                in_=rowmin[:, cb + t0: cb + t1],
                                axis=AX, op=MIN,
                            )
                    # ---- finish direction
                    nco = dd["ncol"]
                    rm = rowmin[:, cb: cb + nco]
                    nc.vector.tensor_tensor(out=rm, in0=rm,
                                            in1=q2_t[:, cb: cb + nco], op=ADD)
                    nc.vector.tensor_tensor(out=rm, in0=rm,
                                            in1=mask_t[:, cb: cb + nco], op=MUL)
                    nc.vector.tensor_scalar_max(out=rm, in0=rm, scalar1=EPS)
                    sq = work.tile([128, max(dirs[0]["ncol"], dirs[1]["ncol"])],
                                   F32, tag="sq")
                    nc.scalar.activation(out=sq[:, 0:nco], in_=rm,
                                         func=mybir.ActivationFunctionType.Sqrt,
                                         accum_out=sums[:, di: di + 1])
                fin = ps.tile([1, 2], F32, tag="d")
                nc.tensor.matmul(fin[0:1, 0:2], ones_t[:, 0:1], sums[:, 0:2],
                                 start=True, stop=True)
                out_sb = work.tile([1, 2], F32, tag="out_sb")
                nc.vector.tensor_copy(out=out_sb[0:1, :], in_=fin[0:1, :])
                nc.sync.dma_start(out_d[:, :], out_sb[0:1, :])
    nc.compile()
    return nc


# ---------------------------------------------------------------- entry point
_CACHE = {}


def _run(inputs, repeat=1):
    from concourse.bass_utils import run_bass_kernel_spmd

    in_maps, dirs = _prepare(inputs["xyz1"], inputs["xyz2"])
    key = (_schedule_key(dirs), repeat)
    if key not in _CACHE:
        _CACHE[key] = _build_nc(dirs, repeat=repeat)
    nc = _CACHE[key]
    res = run_bass_kernel_spmd(nc, in_maps, list(range(8)))
    per_batch = []
    for c in range(B):
        s0, s1 = res.results[c]["out"][0]
        per_batch.append((float(s0) + float(s1)) / (2.0 * N))
    return np.float32(np.mean(per_batch))


def kernel(xyz1, xyz2):
    return _run({"xyz1": xyz1, "xyz2": xyz2}, repeat=1)


# revision 5
# speedup vs baseline: 96.5399x; 96.5399x over previous
"""Chamfer distance (mean of sqrt of min squared distances, both directions) on 8
Trainium2 NeuronCores.

Strategy
--------
Data-parallel over the batch dim: core b handles batch b (B=8, one batch per core).

Per batch, both clouds are sorted (host) along coordinate 0. For each query point a
certified NN search window into the sorted target cloud is computed on host via a
witness bound: r_i = distance to the best of 64 z-nearest candidates; any target
with |z_t - z_q| > r_i is provably farther than the witness, so the true NN lies
inside the window. Points with extreme windows (> threshold T, chosen by exact
cost search on the data) are gathered into separate "overflow" blocks. Window
extents are unioned per block and across batches (one compiled program serves all
cores). Each block's window is covered by 512-wide chunks, grouped <=4 chunks per
PSUM tile; multi-group blocks get temp result columns reduced at the end.

On device, squared distances are computed on the TensorEngine as K=16 matmuls in
bf16 hi/lo split form (abs error ~1e-6):
    g[m,n] = -2*q_m . t_n + |t_n|^2   (4-way bf16 split of |t|^2; all norms from
                                       the bf16-reconstructed points, so g + |q|^2
                                       is exactly |q_hat - t_hat|^2)
    d[m,n] = g[m,n] + |q_m|^2         (query norm added post-min in fp32)
Row minima are reduced on the VectorEngine straight out of PSUM; sqrt runs on the
ScalarEngine with fused row-sum accumulation; a ones-vector matmul does the final
partition sum. Host averages the 16 per-core sums.
"""

import numpy as np
import ml_dtypes

bf = ml_dtypes.bfloat16

B, N, D = 8, 8192, 3
BLK = 128             # query block (matmul M)
CH = 512              # window chunk (matmul N)
NBLK = N // BLK       # 64 regular blocks
GRP = 4               # max chunks per PSUM tile / virtual group
C_WIT = 64            # witness candidates
EPS = 1e-12
T_GRID = (768, 1024, 1536, 2048, 3072, 4096, 6144, 9000)


# ---------------------------------------------------------------- host-side prep
def _splitk(a, k):
    out = []
    r = np.asarray(a, np.float64)
    for _ in range(k):
        h = r.astype(bf)
        out.append(h)
        r = r - h.astype(np.float64)
    return out


def _build_LR(p):
    """p [n,3] f32 sorted -> L [16,n] bf16, R [16,n] bf16, q2 [n] f32."""
    p64 = p.astype(np.float64)
    ph, pl = _splitk(p64, 2)
    phat = ph.astype(np.float64) + pl.astype(np.float64)
    m2h = [(-2.0 * ph[:, c].astype(np.float64)).astype(bf) for c in range(3)]
    m2l = [(-2.0 * pl[:, c].astype(np.float64)).astype(bf) for c in range(3)]
    t2 = (phat ** 2).sum(1)
    t2s = _splitk(t2, 4)
    one = np.ones(p.shape[0], bf)
    L = np.stack(m2h + m2h + m2l + m2l + [one, one, one, one], 0)
    R = np.stack([ph[:, 0], ph[:, 1], ph[:, 2], pl[:, 0], pl[:, 1], pl[:, 2],
                  ph[:, 0], ph[:, 1], ph[:, 2], pl[:, 0], pl[:, 1], pl[:, 2],
                  t2s[0], t2s[1], t2s[2], t2s[3]], 0)
    return L, R, t2.astype(np.float32)


def _pp_windows(q, t):
    """Certified per-point NN windows of q into sorted t -> (lo, hi) int64."""
    zq = q[:, 0].astype(np.float64)
    pos = np.searchsorted(t[:, 0], q[:, 0])
    lo_c = np.clip(pos - C_WIT // 2, 0, N - C_WIT)
    idx = lo_c[:, None] + np.arange(C_WIT)[None, :]
    d = ((q[:, None, :].astype(np.float64) - t[idx].astype(np.float64)) ** 2).sum(-1)
    r = np.sqrt(d.min(1)) * (1 + 1e-6) + 1e-7
    lo = np.searchsorted(t[:, 0], zq - r, side="left")
    hi = np.searchsorted(t[:, 0], zq + r, side="right")
    return lo, hi


def _chunks_for(lo, hi):
    """512-aligned chunk starts covering [lo, hi), clamped into [0, N-CH]."""
    lo = int(min(max(lo, 0), N - 1))
    hi = int(max(hi, lo + 1))
    nch = max(1, -(-(hi - lo) // CH))
    return [min(lo + CH * c, N - CH) for c in range(nch)]


def _schedule_dir(LOd, HId, T):
    """LOd/HId: [B, N] per-point windows for one direction.
    Returns (blocks, nob, cols) where blocks is a list over block_id of
    dicts {groups: [[starts...]...], final_col, temp_cols(range)} and the
    gathered overflow index arrays per batch (ovf_idx [B, nob*BLK])."""
    WID = HId - LOd
    ovf = WID > T                              # [B, N]
    nob = int(np.ceil(max(1, ovf.sum(1).max()) / BLK)) if ovf.any() else 0

    # gathered overflow membership, padded with last real (or 0)
    ovf_idx = np.zeros((B, max(nob, 0) * BLK), np.int64)
    ovf_cnt = np.zeros(B, np.int64)
    for b in range(B):
        ix = np.where(ovf[b])[0]
        ovf_cnt[b] = len(ix)
        if nob:
            padv = ix[-1] if len(ix) else 0
            pad = np.full(nob * BLK, padv, np.int64)
            pad[: len(ix)] = ix
            ovf_idx[b] = pad

    nblk_tot = NBLK + nob
    # per-block unions across batches
    uni = []
    for k in range(NBLK):
        sl = slice(k * BLK, (k + 1) * BLK)
        lo_m = np.where(ovf[:, sl], N, LOd[:, sl]).min()
        hi_m = np.where(ovf[:, sl], 0, HId[:, sl]).max()
        uni.append((lo_m, hi_m))
    for o in range(nob):
        sl = slice(o * BLK, (o + 1) * BLK)
        lo_m, hi_m = N, 0
        for b in range(B):
            ix = ovf_idx[b, sl]
            lo_m = min(lo_m, LOd[b, ix].min())
            hi_m = max(hi_m, HId[b, ix].max())
        uni.append((lo_m, hi_m))

    blocks = []
    ntemp = 0
    for bid in range(nblk_tot):
        starts = _chunks_for(*uni[bid])
        groups = [starts[i: i + GRP] for i in range(0, len(starts), GRP)]
        blocks.append({"groups": groups})
    # column layout: finals [0, nblk_tot), temps afterwards
    tc = nblk_tot
    for blk in blocks:
        g = len(blk["groups"])
        if g == 1:
            blk["temp"] = None
        else:
            blk["temp"] = (tc, tc + g)
            tc += g
    ncol = tc
    return blocks, nob, ncol, ovf_idx, ovf_cnt, ovf


def _dir_cost(blocks):
    cols = 0
    ops = 0
    for blk in blocks:
        for g in blk["groups"]:
            cols += len(g) * CH
            ops += 1
        if blk["temp"] is not None:
            ops += 1
    return cols + 170 * ops


def _prepare(xyz1, xyz2):
    xs_l, ys_l = [], []
    for b in range(B):
        x = np.asarray(xyz1[b], np.float32)
        y = np.asarray(xyz2[b], np.float32)
        xs_l.append(x[np.argsort(x[:, 0], kind="stable")])
        ys_l.append(y[np.argsort(y[:, 0], kind="stable")])

    LO = np.zeros((B, 2, N), np.int64)
    HI = np.zeros((B, 2, N), np.int64)
    for b in range(B):
        for di, (q, t) in enumerate([(xs_l[b], ys_l[b]), (ys_l[b], xs_l[b])]):
            lo, hi = _pp_windows(q, t)
            LO[b, di], HI[b, di] = lo, hi

    dirs = []
    for di in range(2):
        best = None
        for T in T_GRID:
            cand = _schedule_dir(LO[:, di], HI[:, di], T)
            c = _dir_cost(cand[0])
            if best is None or c < best[0]:
                best = (c, T, cand)
        _, T, (blocks, nob, ncol, ovf_idx, ovf_cnt, ovf) = best
        dirs.append({"T": T, "blocks": blocks, "nob": nob, "ncol": ncol,
                     "ovf_idx": ovf_idx, "ovf_cnt": ovf_cnt, "ovf": ovf})

    # ---- per-core tensors
    in_maps = []
    for b in range(B):
        Lx, Rx, q2x = _build_LR(xs_l[b])
        Ly, Ry, q2y = _build_LR(ys_l[b])
        aug_parts = [Lx, Ly, Ry, Rx]
        q2_parts, mask_parts = [], []
        for di in range(2):
            dd = dirs[di]
            q2q = (q2x, q2y)[di]
            Lq = (Lx, Ly)[di]
            nob, ncol = dd["nob"], dd["ncol"]
            if nob:
                aug_parts.append(Lq[:, dd["ovf_idx"][b]])       # [16, nob*BLK]
            q2c = np.zeros((BLK, ncol), np.float32)
            mkc = np.zeros((BLK, ncol), np.float32)
            q2c[:, :NBLK] = q2q.reshape(NBLK, BLK).T
            mkc[:, :NBLK] = (~dd["ovf"][b]).astype(np.float32).reshape(NBLK, BLK).T
            for o in range(nob):
                ix = dd["ovf_idx"][b, o * BLK: (o + 1) * BLK]
                q2c[:, NBLK + o] = q2q[ix]
                slot = o * BLK + np.arange(BLK)
                mkc[:, NBLK + o] = (slot < dd["ovf_cnt"][b]).astype(np.float32)
            q2_parts.append(q2c)
            mask_parts.append(mkc)
        aug = np.concatenate(aug_parts, 1).astype(bf)
        in_maps.append({"aug": np.ascontiguousarray(aug),
                        "q2": np.ascontiguousarray(
                            np.concatenate(q2_parts, 1).astype(np.float32)),
                        "mask": np.ascontiguousarray(
                            np.concatenate(mask_parts, 1).astype(np.float32))})
    return in_maps, dirs


# ---------------------------------------------------------------- device kernel
def _schedule_key(dirs):
    key = []
    for dd in dirs:
        key.append((dd["nob"], dd["ncol"],
                    tuple(tuple(tuple(g) for g in blk["groups"]) +
                          (blk["temp"],) for blk in dd["blocks"])))
    return tuple(key)


def _build_nc(dirs, repeat=1, hw_loop=False):
    import contextlib
    import concourse.bacc as bacc
    import concourse.tile as tile
    import concourse.mybir as mybir

    F32 = mybir.dt.float32
    BF16 = mybir.dt.bfloat16
    AX = mybir.AxisListType.X
    MIN = mybir.AluOpType.min
    ADD = mybir.AluOpType.add
    MUL = mybir.AluOpType.mult

    K = 16
    LBASE = [0, N]
    RBASE = [2 * N, 3 * N]
    OBASE = [4 * N, 4 * N + BLK * dirs[0]["nob"]]
    AUGW = 4 * N + BLK * (dirs[0]["nob"] + dirs[1]["nob"])
    NCOL = dirs[0]["ncol"] + dirs[1]["ncol"]
    CBASE = [0, dirs[0]["ncol"]]

    nc = bacc.Bacc("TRN2", target_bir_lowering=False, debug=False)
    aug_d = nc.dram_tensor("aug", [K, AUGW], BF16, kind="ExternalInput").ap()
    q2_d = nc.dram_tensor("q2", [BLK, NCOL], F32, kind="ExternalInput").ap()
    mask_d = nc.dram_tensor("mask", [BLK, NCOL], F32, kind="ExternalInput").ap()
    out_d = nc.dram_tensor("out", [1, 2], F32, kind="ExternalOutput").ap()

    with tile.TileContext(nc) as tc:
        with (
            tc.tile_pool(name="cst", bufs=1) as cst,
            tc.tile_pool(name="work", bufs=2) as work,
            tc.tile_pool(name="ps", bufs=2, space="PSUM") as ps,
        ):
            aug_t = cst.tile([128, AUGW], BF16)
            NDMA = 8
            step = -(-AUGW // NDMA)
            for i in range(NDMA):
                s = i * step
                e = min(AUGW, s + step)
                if s < e:
                    nc.sync.dma_start(aug_t[0:K, s:e], aug_d[:, s:e])
            q2_t = cst.tile([128, NCOL], F32)
            nc.sync.dma_start(q2_t[:, :], q2_d[:, :])
            mask_t = cst.tile([128, NCOL], F32)
            nc.sync.dma_start(mask_t[:, :], mask_d[:, :])
            ones_t = cst.tile([128, 1], F32)
            nc.vector.memset(ones_t, 1.0)

            if hw_loop:
                rep_iter = [0]
                loop_cm = tc.For_i(0, repeat, 1)
            else:
                rep_iter = range(repeat)
                loop_cm = contextlib.nullcontext()
            with loop_cm:
              for _rep in rep_iter:
                rowmin = work.tile([128, NCOL], F32, tag="rowmin")
                sums = work.tile([128, 2], F32, tag="sums")
                for di in range(2):
                    dd = dirs[di]
                    cb = CBASE[di]
                    for bid, blk in enumerate(dd["blocks"]):
                        if bid < NBLK:
                            lhs = aug_t[0:K, LBASE[di] + BLK * bid:
                                        LBASE[di] + BLK * (bid + 1)]
                        else:
                            o = bid - NBLK
                            lhs = aug_t[0:K, OBASE[di] + BLK * o:
                                        OBASE[di] + BLK * (o + 1)]
                        groups = blk["groups"]
                        for gi, starts in enumerate(groups):
                            nch = len(starts)
                            dps = ps.tile([128, GRP * CH], F32, tag="d")
                            for c, s in enumerate(starts):
                                nc.tensor.matmul(
                                    dps[:, c * CH:(c + 1) * CH],
                                    lhs,
                                    aug_t[0:K, RBASE[di] + s: RBASE[di] + s + CH],
                                    start=True, stop=True,
                                )
                            if blk["temp"] is None:
                                ocol = cb + bid
                            else:
                                ocol = cb + blk["temp"][0] + gi
                            nc.vector.tensor_reduce(
                                out=rowmin[:, ocol: ocol + 1],
                                in_=dps[:, 0:nch * CH], axis=AX, op=MIN,
                            )
                        if blk["temp"] is not None:
                            t0, t1 = blk["temp"]
                            nc.vector.tensor_reduce(
                                out=rowmin[:, cb + bid: cb + bid + 1],
                                in_=rowmin[:, cb + t0: cb + t1],
                                axis=AX, op=MIN,
                            )
                    # ---- finish direction
                    nco = dd["ncol"]
                    rm = rowmin[:, cb: cb + nco]
                    nc.vector.tensor_tensor(out=rm, in0=rm,
                                            in1=q2_t[:, cb: cb + nco], op=ADD)
                    nc.vector.tensor_tensor(out=rm, in0=rm,
                                            in1=mask_t[:, cb: cb + nco], op=MUL)
                    nc.vector.tensor_scalar_max(out=rm, in0=rm, scalar1=EPS)
                    sq = work.tile([128, max(dirs[0]["ncol"], dirs[1]["ncol"])],
                                   F32, tag="sq")
                    nc.scalar.activation(out=sq[:, 0:nco], in_=rm,
                                         func=mybir.ActivationFunctionType.Sqrt,
                                         accum_out=sums[:, di: di + 1])
                fin = ps.tile([1, 2], F32, tag="d")
                nc.tensor.matmul(fin[0:1, 0:2], ones_t[:, 0:1], sums[:, 0:2],
                                 start=True, stop=True)
                out_sb = work.tile([1, 2], F32, tag="out_sb")
                nc.vector.tensor_copy(out=out_sb[0:1, :], in_=fin[0:1, :])
                nc.sync.dma_start(out_d[:, :], out_sb[0:1, :])
    nc.compile()
    return nc


# ---------------------------------------------------------------- entry point
_CACHE = {}


def _run(inputs, repeat=1, hw_loop=False):
    from concourse.bass_utils import run_bass_kernel_spmd

    in_maps, dirs = _prepare(inputs["xyz1"], inputs["xyz2"])
    key = (_schedule_key(dirs), repeat, hw_loop)
    if key not in _CACHE:
        _CACHE[key] = _build_nc(dirs, repeat=repeat, hw_loop=hw_loop)
    nc = _CACHE[key]
    res = run_bass_kernel_spmd(nc, in_maps, list(range(8)))
    per_batch = []
    for c in range(B):
        s0, s1 = res.results[c]["out"][0]
        per_batch.append((float(s0) + float(s1)) / (2.0 * N))
    return np.float32(np.mean(per_batch))


def kernel(xyz1, xyz2):
    return _run({"xyz1": xyz1, "xyz2": xyz2}, repeat=1)
